# revision 19
# baseline (speedup 1.0000x reference)
"""Trainium2 Bass kernel for FlashQwen2Attention (B=4, S=1024, H=4096, 32 q / 8 kv heads).

Sharding: DP over batch (4 groups) x TP over heads (2-way) = 8 cores.
Core c: batch = c // 2, head-half = c % 2 (16 q heads + 4 kv heads per core).
All matmuls run as float32r (full-rate fp32 on the PE, ~1e-4 rel err).
Attention is computed in S^T orientation (scores^T = K^T-chunks x Q^T) so
softmax/exp does the PSUM->SBUF eviction and no PE transposes are needed.
o_proj partials are summed with a ReduceScatter over core pairs.
"""
import sys
import math

sys.path.insert(0, "/opt/trn_rl_repo")

import numpy as np

import concourse.bass as bass
import concourse.mybir as mybir
from concourse import bacc, tile
from concourse.bass_utils import run_bass_kernel_spmd

# ---- problem constants (hardcoded per contract) ----
HIDDEN = 4096
NUM_HEADS = 32
NUM_KV_HEADS = 8
HEAD_DIM = 128
ROPE_THETA = 10000.0
B = 4
S = 1024
P = 128

# per-core shard sizes
NH = 16          # local q heads
NKV = 4          # local kv heads
TC = S           # tokens per core (one batch)
QK_ROWS = NH * HEAD_DIM + NKV * HEAD_DIM   # 2560
V_COLS = NKV * HEAD_DIM                    # 512
QKV_COLS = QK_ROWS + V_COLS                # 3072
F_LOC = NH * HEAD_DIM                      # 2048 (o_proj contraction)
HC = HIDDEN // P                           # 32 hidden chunks
QK_CB = QK_ROWS // P                       # 20 c-blocks
SCALE = 1.0 / math.sqrt(HEAD_DIM)

F32 = mybir.dt.float32
F32R = mybir.dt.float32r


def r(ap):
    return ap


def build_program(single_core=False):
    """single_core=True builds the identical per-core program minus the
    ReduceScatter (for NTFF profiling, which only works on 1 core)."""
    nc = bacc.Bacc("TRN2", target_bir_lowering=False, debug=False,
                   num_devices=1 if single_core else 8)

    hsT = nc.declare_dram_parameter("hsT", [HIDDEN, TC], F32R, isOutput=False)
    wqkvT = nc.declare_dram_parameter("wqkvT", [HIDDEN, QKV_COLS], F32R, isOutput=False)
    bqk = nc.declare_dram_parameter("bqk", [QK_CB, P], F32, isOutput=False)
    bv = nc.declare_dram_parameter("bv", [P, V_COLS], F32, isOutput=False)
    woT = nc.declare_dram_parameter("woT", [F_LOC, HIDDEN], F32R, isOutput=False)
    cos2 = nc.declare_dram_parameter("cos2", [P, TC], F32, isOutput=False)
    sinpm = nc.declare_dram_parameter("sinpm", [P, TC], F32, isOutput=False)
    maskd = nc.declare_dram_parameter("maskd", [P, P], F32, isOutput=False)
    onesp = nc.declare_dram_parameter("onesp", [P, 1], F32R, isOutput=False)
    mask2 = nc.declare_dram_parameter("mask2", [P, 2 * P], F32, isOutput=False)
    out = nc.declare_dram_parameter("out", [S // 2, HIDDEN], F32, isOutput=True)

    qkT_d = nc.dram_tensor("qkT_d", [QK_ROWS, TC], F32)
    v_d = nc.dram_tensor("v_d", [S, V_COLS], F32R)
    oacc = nc.dram_tensor("oacc", [TC, HIDDEN], F32)
    ored = nc.dram_tensor("ored", [TC // 2, HIDDEN], F32)

    with tile.TileContext(nc) as tc:
        cpool_cm = tc.tile_pool(name="consts", bufs=1)
        cpool = cpool_cm.__enter__()
        cos2_sb = cpool.tile([P, TC], F32)
        sinpm_sb = cpool.tile([P, TC], F32)
        maskd_sb = cpool.tile([P, P], F32)
        mask2_sb = cpool.tile([P, 2 * P], F32)
        bqk_sb = cpool.tile([P, QK_CB], F32)
        bv_sb = cpool.tile([P, V_COLS], F32)
        ones_sb = cpool.tile([P, 1], F32R)
        nc.sync.dma_start(cos2_sb[:], cos2[:])
        nc.sync.dma_start(sinpm_sb[:], sinpm[:])
        nc.sync.dma_start(maskd_sb[:], maskd[:])
        nc.sync.dma_start(mask2_sb[:], mask2[:])
        nc.sync.dma_start(bqk_sb[:], bqk.rearrange("cb p -> p cb"))
        nc.sync.dma_start(bv_sb[:], bv[:])
        nc.sync.dma_start(ones_sb[:], onesp[:])

        # ============ stage 1: qkv projection ============
        with (
            tc.tile_pool(name="s1hs", bufs=1) as hs_pool,
            tc.tile_pool(name="s1w", bufs=3) as w_pool,
            tc.tile_pool(name="s1ev", bufs=4) as ev_pool,
        ):
            hs_sb = []
            hsT_r = hsT.rearrange("(hc p) t -> hc p t", p=P)
            for hc in range(HC):
                t_ = hs_pool.tile([P, TC], F32R, tag=f"hs{hc}")
                nc.sync.dma_start(t_[:], hsT_r[hc])
                hs_sb.append(t_)

            wq_r = wqkvT.rearrange("(hc p) c -> hc p c", p=P)
            # q/k sections -> qkT_d (transposed layout [c, t])
            ps_cm = tc.tile_pool(name="s1ps", bufs=4, space="PSUM")
            ps_pool = ps_cm.__enter__()
            for cb in range(QK_CB):
                wts = []
                for hc in range(HC):
                    wt = w_pool.tile([P, P], F32R, tag="wqk", bufs=HC + 16,
                                     name=f"wqk{hc}")
                    nc.sync.dma_start(wt[:], wq_r[hc, :, cb * P:(cb + 1) * P])
                    wts.append(wt)
                for tch in range(2):
                    ps = ps_pool.tile([P, 512], F32, tag="ps1")
                    for hc in range(HC):
                        nc.tensor.matmul(
                            ps[:], r(wts[hc][:]),
                            r(hs_sb[hc][:, tch * 512:(tch + 1) * 512]),
                            start=(hc == 0), stop=(hc == HC - 1),
                        )
                    ev = ev_pool.tile([P, 512], F32, tag="ev1")
                    nc.scalar.activation(
                        ev[:], ps[:], mybir.ActivationFunctionType.Identity,
                        bias=bqk_sb[:, cb:cb + 1],
                    )
                    nc.sync.dma_start(
                        qkT_d[cb * P:(cb + 1) * P, tch * 512:(tch + 1) * 512], ev[:]
                    )

            ps_cm.__exit__(None, None, None)

            # v section, natural orientation [t, v-col]
            with tc.tile_pool(name="s1psv", bufs=1, space="PSUM") as psv_pool:
                v_ps = [psv_pool.tile([P, V_COLS], F32, tag=f"psv{tb}", name=f"psv{tb}")
                        for tb in range(S // P)]
                for hc in range(HC):
                    wv = w_pool.tile([P, V_COLS], F32R, tag="wv")
                    nc.sync.dma_start(wv[:], wq_r[hc, :, QK_ROWS:QKV_COLS])
                    for tb in range(S // P):
                        nc.tensor.matmul(
                            v_ps[tb][:], r(hs_sb[hc][:, tb * P:(tb + 1) * P]),
                            r(wv[:]),
                            start=(hc == 0), stop=(hc == HC - 1),
                        )
                for tb in range(S // P):
                    vev = ev_pool.tile([P, V_COLS], F32R, tag="vev")
                    nc.vector.tensor_tensor(
                        vev[:], v_ps[tb][:], bv_sb[:],
                        mybir.AluOpType.add,
                    )
                    nc.sync.dma_start(v_d[tb * P:(tb + 1) * P, :], vev[:])

        # ============ stage 2: attention ============
        def rope(pool, dst, src):
            xs = pool.tile([P, TC], F32, tag="rope_xs")
            nc.sync.dma_start(xs[0:64, :], src[64:128, :])
            nc.sync.dma_start(xs[64:128, :], src[0:64, :])
            y = pool.tile([P, TC], F32, tag="rope_y")
            nc.vector.tensor_tensor(y[:], src[:], cos2_sb[:], mybir.AluOpType.mult)
            nc.vector.tensor_tensor(xs[:], xs[:], sinpm_sb[:], mybir.AluOpType.mult)
            nc.vector.tensor_tensor(dst[:], y[:], xs[:], mybir.AluOpType.add)

        attnT_cm = tc.tile_pool(name="attnTp", bufs=1)
        attnT_pool = attnT_cm.__enter__()
        attnT = attnT_pool.tile([P, NH, TC], F32R)
        v_res = attnT_pool.tile([P, S // P, V_COLS], F32R)
        nc.sync.dma_start(v_res[:], v_d.rearrange("(tb p) c -> p tb c", p=P))

        with (
            tc.tile_pool(name="s2q", bufs=2) as q_pool,
            tc.tile_pool(name="s2k", bufs=2) as k_pool,
            tc.tile_pool(name="s2pt", bufs=3) as pt_pool,
            tc.tile_pool(name="s2sm", bufs=4) as sm_pool,
            tc.tile_pool(name="s2ps", bufs=2, space="PSUM") as st_ps_pool,
            tc.tile_pool(name="s2po", bufs=2, space="PSUM") as ot_ps_pool,
            tc.tile_pool(name="s2pu", bufs=2, space="PSUM") as su_ps_pool,
        ):
            for g in range(NKV):
                kraw = k_pool.tile([P, TC], F32, tag="kraw")
                nc.sync.dma_start(
                    kraw[:], qkT_d[F_LOC + g * P:F_LOC + (g + 1) * P, :]
                )
                ktr = k_pool.tile([P, TC], F32R, tag="ktr")
                rope(k_pool, ktr[:], kraw[:])
                for hq in range(4):
                    h = 4 * g + hq
                    qraw = q_pool.tile([P, TC], F32, tag="qraw")
                    nc.sync.dma_start(qraw[:], qkT_d[h * P:(h + 1) * P, :])
                    qtr = q_pool.tile([P, TC], F32R, tag="qtr")
                    rope(q_pool, qtr[:], qraw[:])
                    for p in range(4):  # pair of q-blocks 2p, 2p+1
                        nch = 2 * p + 2
                        qs = qtr[:, p * 256:(p + 1) * 256]
                        PT = pt_pool.tile([P, 8, 256], F32R, tag="pt")
                        # S^T chunks in groups of <=4 per psum tile
                        for base in range(0, nch, 4):
                            cnt = min(4, nch - base)
                            st = st_ps_pool.tile([P, 4, 256], F32, tag="st")
                            for i in range(cnt):
                                kc = base + i
                                nc.tensor.matmul(
                                    st[:, i, :],
                                    r(ktr[:, kc * P:(kc + 1) * P]), r(qs),
                                    start=True, stop=True,
                                )
                                if kc == 2 * p:
                                    nc.vector.tensor_tensor(
                                        st[:, i, 0:P], st[:, i, 0:P], maskd_sb[:],
                                        mybir.AluOpType.add,
                                    )
                                elif kc == 2 * p + 1:
                                    nc.vector.tensor_tensor(
                                        st[:, i, :], st[:, i, :], mask2_sb[:],
                                        mybir.AluOpType.add,
                                    )
                            nc.scalar.activation(
                                PT[:, base:base + cnt, :], st[:, 0:cnt, :],
                                mybir.ActivationFunctionType.Exp, scale=SCALE,
                            )
                        ot = ot_ps_pool.tile([P, 256], F32, tag="ot")
                        su = su_ps_pool.tile([1, 256], F32, tag="su")
                        for kc in range(nch):
                            nc.tensor.matmul(
                                ot[:], r(v_res[:, kc, g * P:(g + 1) * P]),
                                r(PT[:, kc, :]),
                                start=(kc == 0), stop=(kc == nch - 1),
                            )
                            nc.tensor.matmul(
                                su[:], r(ones_sb[:]), r(PT[:, kc, :]),
                                start=(kc == 0), stop=(kc == nch - 1),
                            )
                        sums = sm_pool.tile([1, 256], F32, tag="sums")
                        nc.vector.tensor_copy(sums[:], su[:])
                        rec = sm_pool.tile([1, 256], F32, tag="rec")
                        nc.vector.reciprocal(rec[:], sums[:])
                        recb = sm_pool.tile([P, 256], F32, tag="recb")
                        nc.gpsimd.partition_broadcast(recb[:], rec[:])
                        nc.vector.tensor_tensor(
                            attnT[:, h, p * 256:(p + 1) * 256], ot[:],
                            recb[:],
                            mybir.AluOpType.mult,
                        )

        # ============ stage 3: o_proj ============
        with (
            tc.tile_pool(name="s3w", bufs=4) as wo_pool,
            tc.tile_pool(name="s3ev", bufs=6) as oev_pool,
            tc.tile_pool(name="s3ps", bufs=1, space="PSUM") as ops_pool,
        ):
            woT_r = woT.rearrange("(fc p) e -> fc p e", p=P)
            for ec in range(HIDDEN // 512):
                ps = [ops_pool.tile([P, 512], F32, tag=f"ops{tb}", name=f"ops{tb}")
                      for tb in range(S // P)]
                for fc in range(NH):
                    wt = wo_pool.tile([P, 512], F32R, tag="wo")
                    nc.sync.dma_start(wt[:], woT_r[fc, :, ec * 512:(ec + 1) * 512])
                    for tb in range(S // P):
                        nc.tensor.matmul(
                            ps[tb][:], r(attnT[:, fc, tb * P:(tb + 1) * P]),
                            r(wt[:]),
                            start=(fc == 0), stop=(fc == NH - 1),
                        )
                for tb in range(S // P):
                    ev = oev_pool.tile([P, 512], F32, tag="oev")
                    nc.scalar.activation(
                        ev[:], ps[tb][:], mybir.ActivationFunctionType.Copy,
                    )
                    nc.sync.dma_start(
                        oacc[tb * P:(tb + 1) * P, ec * 512:(ec + 1) * 512], ev[:]
                    )

        # ============ reduce-scatter over core pairs + output ============
        if single_core:
            nc.sync.dma_start(out[:], oacc[0:S // 2, :])
        else:
            with tc.tile_critical():
                with nc.semaphore("cc_sem") as cc_sem:
                    nc.gpsimd.collective_compute(
                        "ReduceScatter",
                        mybir.AluOpType.add,
                        replica_groups=[[0, 1], [2, 3], [4, 5], [6, 7]],
                        ins=[oacc[:]],
                        outs=[ored[:]],
                    ).then_inc(cc_sem, 1)
                    nc.gpsimd.wait_ge(cc_sem, 1)
            nc.sync.dma_start(out[:], ored[:])

        attnT_cm.__exit__(None, None, None)
        cpool_cm.__exit__(None, None, None)

    nc.compile()
    return nc


_PROGRAM = None


def _get_program():
    global _PROGRAM
    if _PROGRAM is None:
        _PROGRAM = build_program()
    return _PROGRAM


def _host_inputs(hidden_states, wqkv, bqkv, wo):
    """Per-core input maps (host-side shard + layout prep)."""
    hidden_states = np.asarray(hidden_states, dtype=np.float32)
    wqkv = np.asarray(wqkv, dtype=np.float32)
    bqkv = np.asarray(bqkv, dtype=np.float32)
    wo = np.asarray(wo, dtype=np.float32)

    q_dim = NUM_HEADS * HEAD_DIM
    kv_dim = NUM_KV_HEADS * HEAD_DIM

    # rope tables, [d, t] layout with duplicated cos and +/- sin halves
    inv_freq = ROPE_THETA ** (-np.arange(0, HEAD_DIM, 2, dtype=np.float32) / HEAD_DIM)
    ang = np.arange(S, dtype=np.float32)[None, :] * inv_freq[:, None]  # [64, S]
    cos = np.cos(ang).astype(np.float32)
    sin = np.sin(ang).astype(np.float32)
    cos2 = np.concatenate([cos, cos], axis=0)           # [128, S]
    sinpm = np.concatenate([-sin, sin], axis=0)         # [128, S]

    # causal masks in S^T orientation: maskT[k, q] = 0 if k <= q else -inf
    kk = np.arange(P)[:, None]
    qq = np.arange(P)[None, :]
    maskd = np.where(kk <= qq, 0.0, -1e30).astype(np.float32)   # diagonal block
    mask2 = np.concatenate(
        [np.full((P, P), -1e30, dtype=np.float32), maskd], axis=1
    )  # chunk 2p+1: block-A fully masked | block-B diagonal

    in_maps = []
    for c in range(8):
        b = c // 2
        half = c % 2
        qr = slice(2048 * half, 2048 * half + 2048)
        kr = slice(q_dim + 512 * half, q_dim + 512 * half + 512)
        vr = slice(q_dim + kv_dim + 512 * half, q_dim + kv_dim + 512 * half + 512)

        w_shard = np.concatenate([wqkv[qr], wqkv[kr], wqkv[vr]], axis=0)  # [3072, 4096]
        wqkvT_c = np.ascontiguousarray(w_shard.T)                         # [4096, 3072]
        bqk_c = np.concatenate([bqkv[qr], bqkv[kr]]).reshape(QK_CB, P)
        bv_c = np.broadcast_to(bqkv[vr].reshape(1, V_COLS), (P, V_COLS))
        hsT_c = np.ascontiguousarray(hidden_states[b * S:(b + 1) * S].T)  # [4096, 1024]
        woT_c = np.ascontiguousarray(wo[:, 2048 * half:2048 * half + 2048].T)

        in_maps.append({
            "hsT": hsT_c,
            "wqkvT": wqkvT_c,
            "bqk": np.ascontiguousarray(bqk_c),
            "bv": np.ascontiguousarray(bv_c),
            "woT": woT_c,
            "cos2": cos2,
            "sinpm": sinpm,
            "maskd": maskd,
            "onesp": np.ones((P, 1), dtype=np.float32),
            "mask2": mask2,
        })
    return in_maps


def kernel(hidden_states, wqkv, bqkv, wo, batch_size=B, seq_len=S, _trace=False):
    nc = _get_program()
    in_maps = _host_inputs(hidden_states, wqkv, bqkv, wo)
    res = run_bass_kernel_spmd(nc, in_maps, list(range(8)), trace=_trace)

    out = np.empty((B * S, HIDDEN), dtype=np.float32)
    for c in range(8):
        b = c // 2
        half = c % 2
        rows = slice(b * S + half * (S // 2), b * S + (half + 1) * (S // 2))
        out[rows] = res.results[c]["out"]
    if _trace:
        return out, res
    return out


# revision 23
# speedup vs baseline: 1.2245x; 1.2245x over previous
"""Trainium2 Bass kernel for FlashQwen2Attention (B=4, S=1024, H=4096, 32 q / 8 kv heads).

Sharding: DP over batch (4 groups) x TP over heads (2-way) = 8 cores.
Core c: batch = c // 2, head-half = c % 2 (16 q heads + 4 kv heads per core).
All matmuls run as float32r (full-rate fp32 on the PE, ~1e-4 rel err).
Attention is computed in S^T orientation (scores^T = K^T-chunks x Q^T) so
softmax/exp does the PSUM->SBUF eviction and no PE transposes are needed.
o_proj partials are summed with a ReduceScatter over core pairs.
"""
import sys
import math

sys.path.insert(0, "/opt/trn_rl_repo")

import numpy as np

import concourse.bass as bass
import concourse.mybir as mybir
from concourse import bacc, tile
from concourse.bass_utils import run_bass_kernel_spmd

# ---- problem constants (hardcoded per contract) ----
HIDDEN = 4096
NUM_HEADS = 32
NUM_KV_HEADS = 8
HEAD_DIM = 128
ROPE_THETA = 10000.0
B = 4
S = 1024
P = 128

# per-core shard sizes
NH = 16          # local q heads
NKV = 4          # local kv heads
TC = S           # tokens per core (one batch)
QK_ROWS = NH * HEAD_DIM + NKV * HEAD_DIM   # 2560
V_COLS = NKV * HEAD_DIM                    # 512
QKV_COLS = QK_ROWS + V_COLS                # 3072
F_LOC = NH * HEAD_DIM                      # 2048 (o_proj contraction)
HC = HIDDEN // P                           # 32 hidden chunks
QK_CB = QK_ROWS // P                       # 20 c-blocks
SCALE = 1.0 / math.sqrt(HEAD_DIM)

F32 = mybir.dt.float32
F32R = mybir.dt.float32r


def r(ap):
    return ap


def build_program(single_core=False):
    """single_core=True builds the identical per-core program minus the
    ReduceScatter (for NTFF profiling, which only works on 1 core)."""
    nc = bacc.Bacc("TRN2", target_bir_lowering=False, debug=False,
                   num_devices=1 if single_core else 8)

    hsT = nc.declare_dram_parameter("hsT", [HIDDEN, TC], F32R, isOutput=False)
    wqkvT = nc.declare_dram_parameter("wqkvT", [HIDDEN, QKV_COLS], F32R, isOutput=False)
    bqk = nc.declare_dram_parameter("bqk", [QK_CB, P], F32, isOutput=False)
    bv = nc.declare_dram_parameter("bv", [P, V_COLS], F32, isOutput=False)
    woT = nc.declare_dram_parameter("woT", [F_LOC, HIDDEN], F32R, isOutput=False)
    cos2 = nc.declare_dram_parameter("cos2", [P, TC], F32, isOutput=False)
    sinpm = nc.declare_dram_parameter("sinpm", [P, TC], F32, isOutput=False)
    maskd = nc.declare_dram_parameter("maskd", [P, P], F32, isOutput=False)
    onesp = nc.declare_dram_parameter("onesp", [P, 1], F32R, isOutput=False)
    mask2 = nc.declare_dram_parameter("mask2", [P, 2 * P], F32, isOutput=False)
    out = nc.declare_dram_parameter("out", [S // 2, HIDDEN], F32, isOutput=True)

    qkT_d = nc.dram_tensor("qkT_d", [QK_ROWS, TC], F32)
    v_d = nc.dram_tensor("v_d", [S, V_COLS], F32R)
    oacc = nc.dram_tensor("oacc", [TC, HIDDEN], F32)
    ored = nc.dram_tensor("ored", [TC // 2, HIDDEN], F32)

    with tile.TileContext(nc) as tc:
        cpool_cm = tc.tile_pool(name="consts", bufs=1)
        cpool = cpool_cm.__enter__()
        maskd_sb = cpool.tile([P, P], F32)
        mask2_sb = cpool.tile([P, 2 * P], F32)
        bqk_sb = cpool.tile([P, QK_CB], F32)
        bv_sb = cpool.tile([P, V_COLS], F32)
        ones_sb = cpool.tile([P, 1], F32R)
        nc.sync.dma_start(maskd_sb[:], maskd[:])
        nc.sync.dma_start(mask2_sb[:], mask2[:])
        nc.sync.dma_start(bqk_sb[:], bqk.rearrange("cb p -> p cb"))
        nc.sync.dma_start(bv_sb[:], bv[:])
        nc.sync.dma_start(ones_sb[:], onesp[:])

        # ============ stage 1: qkv projection ============
        with (
            tc.tile_pool(name="s1hs", bufs=1) as hs_pool,
            tc.tile_pool(name="s1w", bufs=3) as w_pool,
            tc.tile_pool(name="s1ev", bufs=4) as ev_pool,
        ):
            hsT_g = hsT.rearrange("(hg g p) t -> hg g p t", g=8, p=P)
            wq_g = wqkvT.rearrange("(hg g p) c -> hg p g c", g=8, p=P)

            # hs chunks loaded in 4 groups of 8, interleaved with weight loads
            hs_sb = []
            for hg in range(4):
                t_ = hs_pool.tile([P, 8, TC], F32R, tag=f"hs{hg}", name=f"hs{hg}")
                nc.sync.dma_start(
                    t_[:], hsT_g[hg].rearrange("g p t -> p g t"))
                hs_sb.append(t_)

            def hs_chunk(hc):
                return hs_sb[hc // 8][:, hc % 8, :]

            ps_cm = tc.tile_pool(name="s1ps", bufs=4, space="PSUM")
            ps_pool = ps_cm.__enter__()

            def qk_cblock(cb):
                """one [c=128, t=1024] block of qkT_d, c-block index cb"""
                wg = []
                for hg in range(4):
                    wt = w_pool.tile([P, 8, P], F32R, tag="wqk", bufs=6,
                                     name=f"wqk{cb}_{hg}")
                    nc.sync.dma_start(wt[:], wq_g[hg, :, :, cb * P:(cb + 1) * P])
                    wg.append(wt)
                for tch in range(2):
                    ps = ps_pool.tile([P, 512], F32, tag="ps1")
                    for hc in range(HC):
                        nc.tensor.matmul(
                            ps[:], wg[hc // 8][:, hc % 8, :],
                            hs_chunk(hc)[:, tch * 512:(tch + 1) * 512],
                            start=(hc == 0), stop=(hc == HC - 1),
                        )
                    ev = ev_pool.tile([P, 512], F32, tag="ev1")
                    nc.scalar.activation(
                        ev[:], ps[:], mybir.ActivationFunctionType.Identity,
                        bias=bqk_sb[:, cb:cb + 1],
                    )
                    nc.scalar.dma_start(
                        qkT_d[cb * P:(cb + 1) * P, tch * 512:(tch + 1) * 512], ev[:]
                    )

            # order: k-section first (attention needs K early), first 4 q heads,
            # then v section, then remaining q heads
            for cb in range(NH, QK_CB):
                qk_cblock(cb)
            for cb in range(0, 4):
                qk_cblock(cb)

            ps_cm.__exit__(None, None, None)

            # v section, natural orientation [t, v-col]
            with tc.tile_pool(name="s1psv", bufs=1, space="PSUM") as psv_pool:
                v_ps = [psv_pool.tile([P, V_COLS], F32, tag=f"psv{tb}", name=f"psv{tb}")
                        for tb in range(S // P)]
                wq_g4 = wqkvT.rearrange("(hg g p) c -> hg p g c", g=4, p=P)
                for hg in range(8):
                    wv = w_pool.tile([P, 4, V_COLS], F32R, tag="wv", bufs=2,
                                     name=f"wv{hg}")
                    nc.sync.dma_start(wv[:], wq_g4[hg, :, :, QK_ROWS:QKV_COLS])
                    for gi in range(4):
                        hc = hg * 4 + gi
                        for tb in range(S // P):
                            nc.tensor.matmul(
                                v_ps[tb][:], hs_chunk(hc)[:, tb * P:(tb + 1) * P],
                                wv[:, gi, :],
                                start=(hc == 0), stop=(hc == HC - 1),
                            )
                for tb in range(S // P):
                    vev = ev_pool.tile([P, V_COLS], F32R, tag="vev")
                    nc.vector.tensor_tensor(
                        vev[:], v_ps[tb][:], bv_sb[:],
                        mybir.AluOpType.add,
                    )
                    nc.scalar.dma_start(v_d[tb * P:(tb + 1) * P, :], vev[:])

            ps2_cm = tc.tile_pool(name="s1ps2", bufs=4, space="PSUM")
            ps_pool = ps2_cm.__enter__()
            for cb in range(4, NH):
                qk_cblock(cb)
            ps2_cm.__exit__(None, None, None)

        # ============ stage 2: attention ============
        def rope(pool, dst, src):
            xs = pool.tile([P, TC], F32, tag="rope_xs")
            nc.gpsimd.dma_start(xs[0:64, :], src[64:128, :])
            nc.gpsimd.dma_start(xs[64:128, :], src[0:64, :])
            y = pool.tile([P, TC], F32, tag="rope_y")
            nc.vector.tensor_tensor(y[:], src[:], cos2_sb[:], mybir.AluOpType.mult)
            nc.vector.tensor_tensor(xs[:], xs[:], sinpm_sb[:], mybir.AluOpType.mult)
            nc.vector.tensor_tensor(dst[:], y[:], xs[:], mybir.AluOpType.add)

        attnT_cm = tc.tile_pool(name="attnTp", bufs=1)
        attnT_pool = attnT_cm.__enter__()
        attnT = attnT_pool.tile([P, NH, TC], F32R)
        v_res = attnT_pool.tile([P, S // P, V_COLS], F32R)
        nc.sync.dma_start(v_res[:], v_d.rearrange("(tb p) c -> p tb c", p=P))
        cos2_sb = attnT_pool.tile([P, TC], F32)
        sinpm_sb = attnT_pool.tile([P, TC], F32)
        nc.sync.dma_start(cos2_sb[:], cos2[:])
        nc.sync.dma_start(sinpm_sb[:], sinpm[:])

        with (
            tc.tile_pool(name="s2q", bufs=2) as q_pool,
            tc.tile_pool(name="s2k", bufs=2) as k_pool,
            tc.tile_pool(name="s2pt", bufs=3) as pt_pool,
            tc.tile_pool(name="s2sm", bufs=4) as sm_pool,
            tc.tile_pool(name="s2ps", bufs=2, space="PSUM") as st_ps_pool,
            tc.tile_pool(name="s2po", bufs=2, space="PSUM") as ot_ps_pool,
            tc.tile_pool(name="s2pu", bufs=2, space="PSUM") as su_ps_pool,
        ):
            for g in range(NKV):
                kraw = k_pool.tile([P, TC], F32, tag="kraw")
                nc.sync.dma_start(
                    kraw[:], qkT_d[F_LOC + g * P:F_LOC + (g + 1) * P, :]
                )
                ktr = k_pool.tile([P, TC], F32R, tag="ktr")
                rope(k_pool, ktr[:], kraw[:])
                for hq in range(4):
                    h = 4 * g + hq
                    qraw = q_pool.tile([P, TC], F32, tag="qraw")
                    nc.sync.dma_start(qraw[:], qkT_d[h * P:(h + 1) * P, :])
                    qtr = q_pool.tile([P, TC], F32R, tag="qtr")
                    rope(q_pool, qtr[:], qraw[:])
                    for p in range(4):  # pair of q-blocks 2p, 2p+1
                        nch = 2 * p + 2
                        qs = qtr[:, p * 256:(p + 1) * 256]
                        PT = pt_pool.tile([P, 8, 256], F32R, tag="pt")
                        # S^T chunks in groups of <=4 per psum tile
                        for base in range(0, nch, 4):
                            cnt = min(4, nch - base)
                            st = st_ps_pool.tile([P, 4, 256], F32, tag="st")
                            for i in range(cnt):
                                kc = base + i
                                nc.tensor.matmul(
                                    st[:, i, :],
                                    r(ktr[:, kc * P:(kc + 1) * P]), r(qs),
                                    start=True, stop=True,
                                )
                                if kc == 2 * p:
                                    nc.vector.tensor_tensor(
                                        st[:, i, 0:P], st[:, i, 0:P], maskd_sb[:],
                                        mybir.AluOpType.add,
                                    )
                                elif kc == 2 * p + 1:
                                    nc.vector.tensor_tensor(
                                        st[:, i, :], st[:, i, :], mask2_sb[:],
                                        mybir.AluOpType.add,
                                    )
                            nc.scalar.activation(
                                PT[:, base:base + cnt, :], st[:, 0:cnt, :],
                                mybir.ActivationFunctionType.Exp, scale=SCALE,
                            )
                        ot = ot_ps_pool.tile([P, 256], F32, tag="ot")
                        su = su_ps_pool.tile([1, 256], F32, tag="su")
                        for kc in range(nch):
                            nc.tensor.matmul(
                                ot[:], r(v_res[:, kc, g * P:(g + 1) * P]),
                                r(PT[:, kc, :]),
                                start=(kc == 0), stop=(kc == nch - 1),
                            )
                        for kc in range(nch):
                            nc.tensor.matmul(
                                su[:], r(ones_sb[:]), r(PT[:, kc, :]),
                                start=(kc == 0), stop=(kc == nch - 1),
                            )
                        sums = sm_pool.tile([1, 256], F32, tag="sums")
                        nc.vector.tensor_copy(sums[:], su[:])
                        rec = sm_pool.tile([1, 256], F32, tag="rec")
                        nc.vector.reciprocal(rec[:], sums[:])
                        recb = sm_pool.tile([P, 256], F32, tag="recb")
                        nc.gpsimd.partition_broadcast(recb[:], rec[:])
                        nc.vector.tensor_tensor(
                            attnT[:, h, p * 256:(p + 1) * 256], ot[:],
                            recb[:],
                            mybir.AluOpType.mult,
                        )

        # ============ stage 3: o_proj ============
        with (
            tc.tile_pool(name="s3w", bufs=4) as wo_pool,
            tc.tile_pool(name="s3ev", bufs=6) as oev_pool,
            tc.tile_pool(name="s3ps", bufs=1, space="PSUM") as ops_pool,
        ):
            woT_g = woT.rearrange("(fg g p) e -> fg p g e", g=4, p=P)
            for ec in range(HIDDEN // 512):
                ps = [ops_pool.tile([P, 512], F32, tag=f"ops{tb}", name=f"ops{tb}")
                      for tb in range(S // P)]
                for fg in range(NH // 4):
                    wt = wo_pool.tile([P, 4, 512], F32R, tag="wo")
                    nc.sync.dma_start(
                        wt[:], woT_g[fg, :, :, ec * 512:(ec + 1) * 512])
                    for gi in range(4):
                        fc = fg * 4 + gi
                        for tb in range(S // P):
                            nc.tensor.matmul(
                                ps[tb][:], attnT[:, fc, tb * P:(tb + 1) * P],
                                wt[:, gi, :],
                                start=(fc == 0), stop=(fc == NH - 1),
                            )
                for tb in range(S // P):
                    ev = oev_pool.tile([P, 512], F32, tag="oev")
                    nc.scalar.activation(
                        ev[:], ps[tb][:], mybir.ActivationFunctionType.Copy,
                    )
                    nc.scalar.dma_start(
                        oacc[tb * P:(tb + 1) * P, ec * 512:(ec + 1) * 512], ev[:]
                    )

        # ============ reduce-scatter over core pairs + output ============
        if single_core:
            nc.sync.dma_start(out[:], oacc[0:S // 2, :])
        else:
            with tc.tile_critical():
                with nc.semaphore("cc_sem") as cc_sem:
                    nc.gpsimd.collective_compute(
                        "ReduceScatter",
                        mybir.AluOpType.add,
                        replica_groups=[[0, 1], [2, 3], [4, 5], [6, 7]],
                        ins=[oacc[:]],
                        outs=[ored[:]],
                    ).then_inc(cc_sem, 1)
                    nc.gpsimd.wait_ge(cc_sem, 1)
            nc.sync.dma_start(out[:], ored[:])

        attnT_cm.__exit__(None, None, None)
        cpool_cm.__exit__(None, None, None)

    nc.compile()
    return nc


_PROGRAM = None


def _get_program():
    global _PROGRAM
    if _PROGRAM is None:
        _PROGRAM = build_program()
    return _PROGRAM


def _host_inputs(hidden_states, wqkv, bqkv, wo):
    """Per-core input maps (host-side shard + layout prep)."""
    hidden_states = np.asarray(hidden_states, dtype=np.float32)
    wqkv = np.asarray(wqkv, dtype=np.float32)
    bqkv = np.asarray(bqkv, dtype=np.float32)
    wo = np.asarray(wo, dtype=np.float32)

    q_dim = NUM_HEADS * HEAD_DIM
    kv_dim = NUM_KV_HEADS * HEAD_DIM

    # rope tables, [d, t] layout with duplicated cos and +/- sin halves
    inv_freq = ROPE_THETA ** (-np.arange(0, HEAD_DIM, 2, dtype=np.float32) / HEAD_DIM)
    ang = np.arange(S, dtype=np.float32)[None, :] * inv_freq[:, None]  # [64, S]
    cos = np.cos(ang).astype(np.float32)
    sin = np.sin(ang).astype(np.float32)
    cos2 = np.concatenate([cos, cos], axis=0)           # [128, S]
    sinpm = np.concatenate([-sin, sin], axis=0)         # [128, S]

    # causal masks in S^T orientation: maskT[k, q] = 0 if k <= q else -inf
    kk = np.arange(P)[:, None]
    qq = np.arange(P)[None, :]
    maskd = np.where(kk <= qq, 0.0, -1e30).astype(np.float32)   # diagonal block
    mask2 = np.concatenate(
        [np.full((P, P), -1e30, dtype=np.float32), maskd], axis=1
    )  # chunk 2p+1: block-A fully masked | block-B diagonal

    in_maps = []
    for c in range(8):
        b = c // 2
        half = c % 2
        qr = slice(2048 * half, 2048 * half + 2048)
        kr = slice(q_dim + 512 * half, q_dim + 512 * half + 512)
        vr = slice(q_dim + kv_dim + 512 * half, q_dim + kv_dim + 512 * half + 512)

        w_shard = np.concatenate([wqkv[qr], wqkv[kr], wqkv[vr]], axis=0)  # [3072, 4096]
        wqkvT_c = np.ascontiguousarray(w_shard.T)                         # [4096, 3072]
        bqk_c = np.concatenate([bqkv[qr], bqkv[kr]]).reshape(QK_CB, P)
        bv_c = np.broadcast_to(bqkv[vr].reshape(1, V_COLS), (P, V_COLS))
        hsT_c = np.ascontiguousarray(hidden_states[b * S:(b + 1) * S].T)  # [4096, 1024]
        woT_c = np.ascontiguousarray(wo[:, 2048 * half:2048 * half + 2048].T)

        in_maps.append({
            "hsT": hsT_c,
            "wqkvT": wqkvT_c,
            "bqk": np.ascontiguousarray(bqk_c),
            "bv": np.ascontiguousarray(bv_c),
            "woT": woT_c,
            "cos2": cos2,
            "sinpm": sinpm,
            "maskd": maskd,
            "onesp": np.ones((P, 1), dtype=np.float32),
            "mask2": mask2,
        })
    return in_maps


def kernel(hidden_states, wqkv, bqkv, wo, batch_size=B, seq_len=S, _trace=False):
    nc = _get_program()
    in_maps = _host_inputs(hidden_states, wqkv, bqkv, wo)
    res = run_bass_kernel_spmd(nc, in_maps, list(range(8)), trace=_trace)

    out = np.empty((B * S, HIDDEN), dtype=np.float32)
    for c in range(8):
        b = c // 2
        half = c % 2
        rows = slice(b * S + half * (S // 2), b * S + (half + 1) * (S // 2))
        out[rows] = res.results[c]["out"]
    if _trace:
        return out, res
    return out
